# revision 56
# baseline (speedup 1.0000x reference)
"""Causal multi-head attention on 8 TRN2 NeuronCores.

Problem: B=4, T=2048, d_model=1024, 16 heads x 64. out = softmax(causal(QK^T)/8) V Wo.

Sharding (tensor-parallel heads x data-parallel batch):
  core c -> batch b = c//2, head group g = c%2 (8 heads each).
  Each core computes a partial output  z_g[b] @ Wo[g] : [2048, 1024];
  host sums the two head-group partials per batch.

Per-core kernel (all matmuls bf16 in / fp32 psum accumulate):
  - host passes x[b]^T (d_model on SBUF partitions everywhere)
  - per q-chunk of 512, per head-pair, per k-block of 128:
      scores via row-split tile_position pair (both heads concurrent on PE),
      one merged exp on ACT ([128, 2*(512-ca)]), tri-mask on DVE (diag only),
      V-augmented-with-ones AV matmuls accumulate z^T and the denominator.
    AV emission is skewed one k-block behind scores so the PE never waits
    on the ACT exp.
  - row end: evict U psum -> SBUF bf16 (frees psum), pack denominators;
    chunk end: one Ln+Exp on [8,512] -> 1/D, matmul-broadcast to 64
    partitions, DVE mul -> zt bf16 -> output projection -> DMA out.
  - proj/outproj units are interleaved between attention units by a
    build-time greedy scheduler that tracks simulated PE/ACT clocks.
"""
import numpy as np

import concourse.bass as bass
import concourse.tile as tile
import concourse.mybir as mybir
from concourse.vector_clock import ScopedClock
from concourse.bass_utils import run_bass_kernel_spmd

D_MODEL = 1024
D_HEAD = 64
B = 4
T = 2048
H = 8              # heads per core
HG = H * D_HEAD    # 512 head-dim columns per core
TCH = 512          # q/t chunk
NCH = T // TCH     # 4
NDM = D_MODEL // 128  # 8 d_model chunks

F32 = mybir.dt.float32
BF16 = mybir.dt.bfloat16
AF = mybir.ActivationFunctionType


class _TC(tile.TileContext):
    """TileContext whose tail drain carries no sem waits (this walrus build
    rejects >1 sync wait per instruction and any wait on a Drain)."""

    def _drain_and_barrier(self, tick_clock, wait_clock):
        drain_inst = self.nc.sync.drain()
        wait_clock.add_sem_waits(
            drain_inst.ins, ScopedClock({None: tick_clock.global_clock})
        )
        si = drain_inst.ins.sync_info
        waits = list(si.on_wait) if si is not None else []
        if waits:
            drain_inst.ins.sync_info = mybir.SyncInfo(
                on_wait=[], on_update=list(si.on_update)
            )
            for w in waits:
                nop = self.nc.sync.nop(nofuse=True)
                nop.ins.sync_info = mybir.SyncInfo(on_wait=[w], on_update=[])
        self.nc.all_engine_barrier()
        popped = self.nc._tile_sem_poison_stack.pop()
        assert popped is self._sem_poison
        self.nc.clear_and_free_semaphores(list(self.sems.allocated().values()))
        self.nc.all_engine_barrier()


def _split_multi_waits(nc):
    """Move all-but-one sem wait of every instruction onto same-engine NOPs."""
    cnt = 0
    for f in nc.m.functions:
        for b in f.blocks:
            new = []
            for inst in b.instructions:
                si = inst.sync_info
                if si is not None and si.on_wait is not None:
                    waits = list(si.on_wait)
                    max_keep = 0 if inst.opcode == "Drain" else 1
                    if len(waits) > max_keep:
                        keep = waits[len(waits) - max_keep:] if max_keep else []
                        spill = waits[: len(waits) - max_keep]
                        for w in spill:
                            nop = mybir.InstNoOp(
                                name=f"I-wsplit-{cnt}", engine=inst.engine,
                                ins=[], outs=[],
                            )
                            nop.sync_info = mybir.SyncInfo(
                                on_wait=[w], on_update=[]
                            )
                            new.append(nop)
                            cnt += 1
                        inst.sync_info = mybir.SyncInfo(
                            on_wait=keep, on_update=list(si.on_update)
                        )
                new.append(inst)
            b.instructions = new
    return cnt


def _build():
    nc = bass.Bass("TRN2", target_bir_lowering=False)
    xT = nc.dram_tensor("xT", (D_MODEL, T), BF16, kind="ExternalInput")
    wq = nc.dram_tensor("wq", (D_MODEL, HG), BF16, kind="ExternalInput")
    wk = nc.dram_tensor("wk", (D_MODEL, HG), BF16, kind="ExternalInput")
    wv = nc.dram_tensor("wv", (D_MODEL, HG), BF16, kind="ExternalInput")
    wo = nc.dram_tensor("wo", (HG, D_MODEL), BF16, kind="ExternalInput")
    tri = nc.dram_tensor("tri", (128, 128), BF16, kind="ExternalInput")
    sel8 = nc.dram_tensor("sel8", (8, 8 * 64), BF16, kind="ExternalInput")
    sel65 = nc.dram_tensor("sel65", (D_HEAD + 1, 8 * 8), BF16,
                           kind="ExternalInput")
    vones = nc.dram_tensor("vones", (128, T // 128, H, 1), BF16,
                           kind="ExternalInput")
    out = nc.dram_tensor("out", (T, D_MODEL), F32, kind="ExternalOutput")

    from contextlib import ExitStack
    with _TC(nc) as tc, ExitStack() as ctx:
        consts = ctx.enter_context(tc.tile_pool(name="consts", bufs=1))
        xs_pool = ctx.enter_context(tc.tile_pool(name="xs", bufs=3))
        kt_pool = ctx.enter_context(tc.tile_pool(name="kt", bufs=1))
        v_pool = ctx.enter_context(tc.tile_pool(name="v", bufs=1))
        qt_pool = ctx.enter_context(tc.tile_pool(name="qt", bufs=3))
        zt_pool = ctx.enter_context(tc.tile_pool(name="zt", bufs=2))
        et_pool = ctx.enter_context(tc.tile_pool(name="et", bufs=4))
        ue_pool = ctx.enter_context(tc.tile_pool(name="ue", bufs=8))
        sm_pool = ctx.enter_context(tc.tile_pool(name="sm", bufs=6))
        ou_pool = ctx.enter_context(tc.tile_pool(name="ou", bufs=3))
        # PSUM: scores 2x2 banks, U 2x1, proj/outproj/db 1, den8 1 = 8 banks
        ps_s = ctx.enter_context(tc.tile_pool(name="ps_s", bufs=2, space="PSUM"))
        ps_u = ctx.enter_context(tc.tile_pool(name="ps_u", bufs=2, space="PSUM"))
        ps_w = ctx.enter_context(tc.tile_pool(name="ps_w", bufs=1, space="PSUM"))
        ps_d = ctx.enter_context(tc.tile_pool(name="ps_d", bufs=1, space="PSUM"))

        xT_r = xT.ap().rearrange("(c p) t -> p c t", p=128)

        # resident weights / constants (wq/wk + first x chunk lead: they gate
        # the first matmuls)
        wq_sb = consts.tile([128, NDM, HG], BF16)
        xs0 = xs_pool.tile([128, NDM, TCH], BF16, name="xs", tag="xs")
        wk_sb = consts.tile([128, NDM, HG], BF16)
        wv_sb = consts.tile([128, NDM, HG], BF16)
        # spread the head DMAs over the three DMA-capable queues; xs0 on two
        # rings, wq/wk column-major per head-pair so each uq(dqc)/uk(dqc)
        # only waits for its own slice
        wq_r = wq.ap().rearrange("(c p) n -> p c n", p=128)
        wk_r = wk.ap().rearrange("(c p) n -> p c n", p=128)
        for c in range(NDM):
            eng = nc.sync if c % 2 == 0 else nc.scalar
            eng.dma_start(out=xs0[:, c, :], in_=xT_r[:, c, 0:TCH])
        for dqc in range(4):
            cs = slice(dqc * 128, (dqc + 1) * 128)
            nc.gpsimd.dma_start(out=wq_sb[:, :, cs], in_=wq_r[:, :, cs])
            nc.gpsimd.dma_start(out=wk_sb[:, :, cs], in_=wk_r[:, :, cs])
        wv_r = wv.ap().rearrange("(c p) n -> p c n", p=128)
        for c in range(NDM):
            eng = nc.scalar if c % 2 == 0 else nc.sync
            eng.dma_start(out=wv_sb[:, c, :], in_=wv_r[:, c, :])
        tri_sb = consts.tile([128, 128], BF16)
        nc.sync.dma_start(out=tri_sb, in_=tri.ap())
        sel8_sb = consts.tile([8, 8 * 64], BF16)
        nc.sync.dma_start(out=sel8_sb, in_=sel8.ap())
        sel65_sb = consts.tile([D_HEAD + 1, 8, 8], BF16)
        nc.sync.dma_start(out=sel65_sb,
                          in_=sel65.ap().rearrange("p (g m) -> p g m", g=8))
        wo_sb = consts.tile([128, HG // 128, D_MODEL], BF16)
        nc.gpsimd.dma_start(out=wo_sb,
                            in_=wo.ap().rearrange("(c p) n -> p c n", p=128))
        # per-chunk K^T tiles [pair-packed 128, pair, t-in-chunk] and V tiles
        # (V has a ones column so row 64 of U accumulates the denominator)
        kt_tiles = [kt_pool.tile([128, 4, TCH], BF16, name=f"kt{i}", tag=f"kt{i}")
                    for i in range(NCH)]
        v_tiles = [v_pool.tile([128, 4, H, D_HEAD + 1], BF16, name=f"v{i}",
                               tag=f"v{i}") for i in range(NCH)]
        vo_r = vones.ap().rearrange("p (a b) h o -> p a b h o", b=4)
        for i in range(NCH):
            nc.sync.dma_start(out=v_tiles[i][:, :, :, D_HEAD:], in_=vo_r[:, i])

        # ---------------- unit definitions ----------------
        # Each unit is (cost_pe_ns, cost_act_ns, emit_fn). The scheduler
        # tracks simulated engine clocks and interleaves fill (proj/outproj)
        # between attention units so the PE rides just behind ACT.

        # Fill units are emitted as ~430ns granules (2 matmuls each) so the
        # scheduler can drop them into the short PE holes between attention
        # units.  Granule tag: ('proj', ch, sub, idx) / ('outproj', ch, ...).
        def proj_units(ch, xs, qt_sb):
            units = []
            for dqc in range(4):
                st = {}
                def g0(dqc=dqc, st=st):
                    st['pq'] = ps_w.tile([128, TCH], F32, tag="ps_w", name="pq")
                    for c in range(2):
                        nc.tensor.matmul(
                            st['pq'], lhsT=wq_sb[:, c, dqc * 128:(dqc + 1) * 128],
                            rhs=xs[:, c, :], start=(c == 0), stop=False)
                def gmid(c0, dqc=dqc, st=st):
                    for c in range(c0, c0 + 2):
                        nc.tensor.matmul(
                            st['pq'], lhsT=wq_sb[:, c, dqc * 128:(dqc + 1) * 128],
                            rhs=xs[:, c, :], start=False, stop=False)
                def g3(dqc=dqc, st=st):
                    for c in range(6, NDM):
                        nc.tensor.matmul(
                            st['pq'], lhsT=wq_sb[:, c, dqc * 128:(dqc + 1) * 128],
                            rhs=xs[:, c, :], start=False, stop=(c == NDM - 1))
                    nc.vector.tensor_copy(out=qt_sb[:, dqc, :], in_=st['pq'])
                units += [('proj', ch, 'q', dqc, 440, g0, 'a'),
                          ('proj', ch, 'q', dqc, 440, lambda c0=2, g=gmid: g(c0), 'm'),
                          ('proj', ch, 'q', dqc, 440, lambda c0=4, g=gmid: g(c0), 'm'),
                          ('proj', ch, 'q', dqc, 470, g3, 'e')]
            for dqc in range(4):
                st = {}
                def k0(dqc=dqc, st=st):
                    st['pk'] = ps_w.tile([128, TCH], F32, tag="ps_w", name="pk")
                    for c in range(2):
                        nc.tensor.matmul(
                            st['pk'], lhsT=wk_sb[:, c, dqc * 128:(dqc + 1) * 128],
                            rhs=xs[:, c, :], start=(c == 0), stop=False)
                def kmid(c0, dqc=dqc, st=st):
                    for c in range(c0, c0 + 2):
                        nc.tensor.matmul(
                            st['pk'], lhsT=wk_sb[:, c, dqc * 128:(dqc + 1) * 128],
                            rhs=xs[:, c, :], start=False, stop=False)
                def k3(ch=ch, dqc=dqc, st=st):
                    for c in range(6, NDM):
                        nc.tensor.matmul(
                            st['pk'], lhsT=wk_sb[:, c, dqc * 128:(dqc + 1) * 128],
                            rhs=xs[:, c, :], start=False, stop=(c == NDM - 1))
                    nc.vector.tensor_copy(out=kt_tiles[ch][:, dqc, :],
                                          in_=st['pk'])
                units += [('proj', ch, 'k', dqc, 440, k0, 'a'),
                          ('proj', ch, 'k', dqc, 440, lambda c0=2, g=kmid: g(c0), 'm'),
                          ('proj', ch, 'k', dqc, 440, lambda c0=4, g=kmid: g(c0), 'm'),
                          ('proj', ch, 'k', dqc, 470, k3, 'e')]
            for tt in range(4):
                st = {}
                def v0(tt=tt, st=st):
                    st['pv'] = ps_w.tile([128, HG], F32, tag="ps_w", name="pv")
                    for c in range(2):
                        nc.tensor.matmul(
                            st['pv'], lhsT=xs[:, c, tt * 128:(tt + 1) * 128],
                            rhs=wv_sb[:, c, :], start=(c == 0), stop=False)
                def vmid(c0, tt=tt, st=st):
                    for c in range(c0, c0 + 2):
                        nc.tensor.matmul(
                            st['pv'], lhsT=xs[:, c, tt * 128:(tt + 1) * 128],
                            rhs=wv_sb[:, c, :], start=False, stop=False)
                def v3(ch=ch, tt=tt, st=st):
                    for c in range(6, NDM):
                        nc.tensor.matmul(
                            st['pv'], lhsT=xs[:, c, tt * 128:(tt + 1) * 128],
                            rhs=wv_sb[:, c, :], start=False, stop=(c == NDM - 1))
                    nc.vector.tensor_copy(
                        out=v_tiles[ch][:, tt, :, 0:D_HEAD],
                        in_=st['pv'].rearrange("p (h d) -> p h d", h=H))
                units += [('proj', ch, 'v', tt, 440, v0, 'a'),
                          ('proj', ch, 'v', tt, 440, lambda c0=2, g=vmid: g(c0), 'm'),
                          ('proj', ch, 'v', tt, 440, lambda c0=4, g=vmid: g(c0), 'm'),
                          ('proj', ch, 'v', tt, 470, v3, 'e')]
            return units

        def outproj_units(ch, zt_sb, scalar_evict=False, alt_pool=False):
            units = []
            q0 = ch * TCH
            for tt in range(4):
                st = {}
                def o_alloc(st=st):
                    st['o'] = ou_pool.tile([128, D_MODEL], F32, name="o_sb")
                def mk_half(dc, tt=tt, st=st):
                    def first(dc=dc, tt=tt, st=st):
                        pool, tag = ((ps_d, "den8")
                                     if alt_pool and (tt * 2 + dc) % 2
                                     else (ps_w, "ps_w"))
                        st['po'] = pool.tile([128, 512], F32, tag=tag,
                                             name="po")
                        for kc in range(2):
                            nc.tensor.matmul(
                                st['po'],
                                lhsT=zt_sb[:, kc, tt * 128:(tt + 1) * 128],
                                rhs=wo_sb[:, kc, dc * 512:(dc + 1) * 512],
                                start=(kc == 0), stop=False)
                    def second(dc=dc, tt=tt, st=st):
                        for kc in range(2, 4):
                            nc.tensor.matmul(
                                st['po'],
                                lhsT=zt_sb[:, kc, tt * 128:(tt + 1) * 128],
                                rhs=wo_sb[:, kc, dc * 512:(dc + 1) * 512],
                                start=False, stop=(kc == 3))
                        if scalar_evict:
                            nc.scalar.activation(
                                out=st['o'][:, dc * 512:(dc + 1) * 512],
                                in_=st['po'], func=AF.Copy)
                        else:
                            nc.vector.tensor_copy(
                                out=st['o'][:, dc * 512:(dc + 1) * 512],
                                in_=st['po'])
                        if dc == 1:
                            r0 = q0 + tt * 128
                            eng = nc.sync if tt % 2 == 0 else nc.gpsimd
                            eng.dma_start(out=out.ap()[r0:r0 + 128, :],
                                          in_=st['o'])
                    return first, second
                f0, s0 = mk_half(0)
                f1, s1 = mk_half(1)
                def g00(f=f0, oa=o_alloc):
                    oa(); f()
                units += [('outproj', ch, 'o', tt, 440, g00, 'a'),
                          ('outproj', ch, 'o', tt, 470, s0, 'e'),
                          ('outproj', ch, 'o', tt, 440, f1, 'a'),
                          ('outproj', ch, 'o', tt, 470, s1, 'e')]
            return units

        # ---- attention row (chunk ch, head pair hp) ----
        # units tagged ('S', kb) / ('AV', kb) / ('EV', None) so the emission
        # loop can model the exp(kb) -> AV(kb) gating.
        def att_row_units(ch, hp, qt_sb, uev, den8_ps, row_state):
            nkb = 4 * ch + 4
            ets = [None] * nkb
            u_ps = [None, None]

            def mk_S(kb):
                j = kb - 4 * ch
                ca = 128 * j if j > 0 else 0
                ncols = TCH - ca
                def S(kb=kb, ca=ca, j=j):
                    s2 = ps_s.tile([128, 2, TCH], F32, name="s2", tag="s2")
                    kt_t = kt_tiles[kb // 4]
                    oa = (kb % 4) * 128
                    for par in range(2):
                        p0, p1 = 64 * par, 64 * par + 64
                        nc.tensor.matmul(
                            s2[:, par, ca:],
                            lhsT=kt_t[p0:p1, hp, oa:oa + 128],
                            rhs=qt_sb[p0:p1, hp, ca:],
                            start=True, stop=True,
                            tile_position=(64 * par, 0))
                    et = et_pool.tile([128, 2, TCH], BF16, name="et", tag="et")
                    nc.scalar.activation(out=et[:, :, ca:], in_=s2[:, :, ca:],
                                         func=AF.Exp, scale=0.125)
                    if j >= 0:
                        for par in range(2):
                            nc.vector.tensor_mul(et[:, par, ca:ca + 128],
                                                 et[:, par, ca:ca + 128],
                                                 tri_sb)
                    ets[kb] = et
                # PE: the two score MMs run concurrently (row split)
                return ('S', kb, int(ncols / 2.4) + 15,
                        int((2 * ncols + 352) / 1.2) + 60, S)

            def mk_AV(kb):
                j = kb - 4 * ch
                ca = 128 * j if j > 0 else 0
                ncols = TCH - ca
                def AV(kb=kb, ca=ca):
                    if kb == 0:
                        u_ps[0] = ps_u.tile([D_HEAD + 1, TCH], F32, name="u_ps",
                                            tag="u_ps")
                        u_ps[1] = ps_u.tile([D_HEAD + 1, TCH], F32, name="u_ps",
                                            tag="u_ps")
                    et = ets[kb]
                    for par in range(2):
                        h = 2 * hp + par
                        nc.tensor.matmul(
                            u_ps[par][:, ca:],
                            lhsT=v_tiles[kb // 4][:, kb % 4, h, :],
                            rhs=et[:, par, ca:],
                            start=(kb == 0), stop=(kb == nkb - 1))
                    ets[kb] = None
                return ('AV', kb, 2 * int(ncols / 2.4) + 25, 0, AV)

            def evict():
                # U psum -> SBUF bf16 (frees the 2 psum banks for next row),
                # then matmul-gather this row's denominators (uev row 64).
                # ACT-copy in ch0/1 keeps the DVE queue clear of the next
                # row's gating tri-mask.
                for par in range(2):
                    nc.vector.tensor_copy(out=uev[:, par, :], in_=u_ps[par])
                for par in range(2):
                    g = 2 * hp + par
                    nc.tensor.matmul(den8_ps, lhsT=sel65_sb[:, g, :],
                                     rhs=uev[:, par, :],
                                     start=(g == 0), stop=(g == 7))

            units = []
            units.append(mk_S(0))
            for kb in range(1, nkb):
                units.append(mk_S(kb))
                units.append(mk_AV(kb - 1))
            units.append(mk_AV(nkb - 1))
            units.append(('EV', None, 450, 0, evict))
            return units

        def div_units(ch, uevs, den8_ps, zt_sb):
            """Chunk end: 1/D for all 8 heads, broadcast, zt = U * (1/D)."""
            units = []
            lnd = sm_pool.tile([8, TCH], F32, name="lnd")
            rcp8 = sm_pool.tile([8, TCH], BF16, name="rcp8")
            def u_recip():
                nc.scalar.activation(out=lnd, in_=den8_ps, func=AF.Ln)
                nc.scalar.activation(out=rcp8, in_=lnd, func=AF.Exp,
                                     scale=-1.0)
            units.append((0, 1500, u_recip))
            for hp in range(4):
                def u_div(hp=hp):
                    # separate banks per par (ps_w / ps_d alternate) so the
                    # broadcast of one par overlaps the DVE mul of the other
                    dbs = []
                    for par in range(2):
                        g = 2 * hp + par
                        pool, tag = ((ps_w, "ps_w") if par == 0
                                     else (ps_d, "den8"))
                        db = pool.tile([64, TCH], F32, tag=tag, name="db")
                        nc.tensor.matmul(db,
                                         lhsT=sel8_sb[:, g * 64:(g + 1) * 64],
                                         rhs=rcp8, start=True, stop=True)
                        dbs.append(db)
                    for par in range(2):
                        nc.vector.tensor_mul(
                            zt_sb[64 * par:64 * par + 64, hp, :],
                            uevs[hp][0:D_HEAD, par, :], dbs[par])
                units.append((450, 0, u_div))
            return units

        # ---------------- schedule ----------------
        qt_tiles = [None] * NCH
        xs_tiles = [xs0] + [None] * (NCH - 1)
        zt_tiles = [None] * NCH

        def stage_proj(ch):
            if ch > 0:
                xs_tiles[ch] = xs_pool.tile([128, NDM, TCH], BF16, name="xs",
                                            tag="xs")
                nc.sync.dma_start(
                    out=xs_tiles[ch],
                    in_=xT_r[:, :, ch * TCH:(ch + 1) * TCH])
            qt_tiles[ch] = qt_pool.tile([128, 4, TCH], BF16, name="qt",
                                        tag="qt")
            return proj_units(ch, xs_tiles[ch], qt_tiles[ch])

        # Global fill queue of granules.  'proj' granules required by an
        # attention unit are force-emitted (with their queue prefix) just
        # before it; the rest drop into PE holes (AV exp-waits) or drain
        # when the PE trails ACT.  The scheduler may only pause the queue
        # when no ps_w psum slot is held open by a partially-emitted unit.
        fillq = []
        div_done = [False] * NCH
        pe_t = [0.0]
        act_t = [0.0]
        psw_open = [False]

        def emit_one(ent):
            ent[5]()
            pe_t[0] += ent[4]
            if ent[6] == 'a':
                psw_open[0] = True
            elif ent[6] == 'e':
                psw_open[0] = False

        def emit_fill(gate_t):
            while fillq:
                ent = fillq[0]
                if not psw_open[0] and \
                        pe_t[0] + ent[4] > max(gate_t + 350, act_t[0] + 350):
                    break
                fillq.pop(0)
                emit_one(ent)

        def force_fill(pred):
            """Emit the queue prefix up to the last entry matching pred."""
            last = -1
            for i, ent in enumerate(fillq):
                if pred(ent):
                    last = i
            for ent in fillq[:last + 1]:
                emit_one(ent)
            del fillq[:last + 1]

        def drain_psw():
            while fillq and psw_open[0]:
                emit_one(fillq.pop(0))

        # chunk 0 projections up front, ordered q0,k0 first so the first
        # scores/exp fire as soon as their DMA slices land
        p0 = stage_proj(0)
        def _k0(e):
            if e[2] == 'v':
                return 100 + e[3]          # v last (waits on the wv DMA)
            return e[3] * 2 + (e[2] == 'k')
        for ent in sorted(p0, key=_k0):
            emit_one(ent)

        for ch in range(NCH):
            last = (ch == NCH - 1)
            zt_tiles[ch] = zt_pool.tile([128, 4, TCH], BF16, name="zt",
                                        tag="zt")
            den8_ps = ps_d.tile([8, TCH], F32, name="den8", tag="den8")
            uevs = []
            # queue next chunk's projections as fill
            if ch + 1 < NCH:
                for ent in stage_proj(ch + 1):
                    fillq.append(ent)
            rows_data = []
            for hp in range(4):
                uev = ue_pool.tile([D_HEAD + 1, 2, TCH], BF16, name="uev",
                                   tag="uev")
                uevs.append(uev)
                rows_data.append(att_row_units(ch, hp, qt_tiles[ch], uev,
                                               den8_ps, {}))

            def emit_att(hp, exp_done, tag, kb, cost_pe, cost_act, fn):
                if tag == 'S':
                    force_fill(lambda e, kb=kb, hp=hp: e[0] == 'proj'
                               and e[2] == 'q' and e[1] == ch
                               and e[3] == hp)
                    force_fill(lambda e, kb=kb, hp=hp: e[0] == 'proj'
                               and e[2] == 'k' and e[1] == kb // 4
                               and e[3] == hp)
                    gate = exp_done.get(kb - 2, 0.0)   # s2 pool depth 2
                elif tag == 'AV':
                    force_fill(lambda e, kb=kb: e[0] == 'proj'
                               and e[2] == 'v' and e[1] == kb // 4
                               and e[3] == kb % 4)
                    gate = exp_done.get(kb, 0.0)
                else:
                    gate = 0.0
                emit_fill(gate)
                fn()
                if tag == 'S':
                    pe_t[0] = max(pe_t[0], gate) + cost_pe
                    act_t[0] = max(act_t[0], pe_t[0] + 250) + cost_act
                    exp_done[kb] = act_t[0]
                elif tag == 'AV':
                    pe_t[0] = max(pe_t[0], gate + 250) + cost_pe
                else:
                    pe_t[0] += cost_pe

            # emit rows with the next row's first scores matmul hoisted
            # ahead of this row's evict/gather, so the ACT exp stream never
            # drains across a row boundary
            exp_dones = [dict() for _ in range(4)]
            prefetched = set()
            for hp in range(4):
                units = rows_data[hp]
                start = 1 if hp in prefetched else 0
                for u in units[start:-1]:
                    emit_att(hp, exp_dones[hp], *u)
                if hp + 1 < 4:
                    emit_att(hp + 1, exp_dones[hp + 1], *rows_data[hp + 1][0])
                    prefetched.add(hp + 1)
                emit_att(hp, exp_dones[hp], *units[-1])
            drain_psw()
            for _, cost_act, fn in div_units(ch, uevs, den8_ps,
                                             zt_tiles[ch]):
                emit_fill(pe_t[0])
                drain_psw()
                fn()
                if cost_act:
                    act_t[0] = max(act_t[0], pe_t[0]) + cost_act
                else:
                    pe_t[0] += 450
            div_done[ch] = True
            for ent in outproj_units(ch, zt_tiles[ch], scalar_evict=last,
                                     alt_pool=last):
                fillq.append(ent)
        # drain remaining fill (tail outprojs)
        for ent in list(fillq):
            emit_one(ent)

    _split_multi_waits(nc)
    return nc


_NC_CACHE = None


def _get_nc():
    global _NC_CACHE
    if _NC_CACHE is None:
        _NC_CACHE = _build()
    return _NC_CACHE


def _make_in_maps(x, W_Q, W_K, W_V, W_O):
    x = np.asarray(x, dtype=np.float32)
    W_Q = np.asarray(W_Q, dtype=np.float32)
    W_K = np.asarray(W_K, dtype=np.float32)
    W_V = np.asarray(W_V, dtype=np.float32)
    W_O = np.asarray(W_O, dtype=np.float32)

    import ml_dtypes
    bf = ml_dtypes.bfloat16
    tri = np.triu(np.ones((128, 128), dtype=bf))  # col >= row
    sel8 = np.zeros((8, 8 * 64), dtype=bf)
    for g in range(8):
        sel8[g, g * 64:(g + 1) * 64] = 1.0
    sel65 = np.zeros((D_HEAD + 1, 8 * 8), dtype=bf)
    for g in range(8):
        sel65[D_HEAD, g * 8 + g] = 1.0
    vones = np.ones((128, T // 128, H, 1), dtype=bf)

    in_maps = []
    for core in range(8):
        b, g = core // 2, core % 2
        cs = slice(g * HG, (g + 1) * HG)
        in_maps.append({
            "xT": np.ascontiguousarray(x[b].T).astype(bf),
            "wq": np.ascontiguousarray(W_Q[:, cs]).astype(bf),
            "wk": np.ascontiguousarray(W_K[:, cs]).astype(bf),
            "wv": np.ascontiguousarray(W_V[:, cs]).astype(bf),
            "wo": np.ascontiguousarray(W_O[cs, :]).astype(bf),
            "tri": tri, "sel8": sel8, "sel65": sel65, "vones": vones,
        })
    return in_maps


def kernel(x, W_Q, W_K, W_V, W_O):
    in_maps = _make_in_maps(x, W_Q, W_K, W_V, W_O)
    nc = _get_nc()
    res = run_bass_kernel_spmd(nc, in_maps, core_ids=list(range(8)))
    outs = [res.results[c]["out"] for c in range(8)]
    full = np.stack([outs[2 * b] + outs[2 * b + 1] for b in range(B)], axis=0)
    return full


# revision 59
# speedup vs baseline: 1.1850x; 1.1850x over previous
"""Causal multi-head attention on 8 TRN2 NeuronCores.

Problem: B=4, T=2048, d_model=1024, 16 heads x 64. out = softmax(causal(QK^T)/8) V Wo.

Sharding (tensor-parallel heads x data-parallel batch):
  core c -> batch b = c//2, head group g = c%2 (8 heads each).
  Each core computes a partial output  z_g[b] @ Wo[g] : [2048, 1024];
  host sums the two head-group partials per batch.

Per-core kernel (all matmuls bf16 in / fp32 psum accumulate):
  - host passes x[b]^T (d_model on SBUF partitions everywhere)
  - per q-chunk of 512, per head-pair, per k-block of 128:
      scores via row-split tile_position pair (both heads concurrent on PE),
      one merged exp on ACT ([128, 2*(512-ca)]), tri-mask on DVE (diag only),
      V-augmented-with-ones AV matmuls accumulate z^T and the denominator.
    AV emission is skewed one k-block behind scores so the PE never waits
    on the ACT exp.
  - row end: evict U psum -> SBUF bf16 (frees psum), pack denominators;
    chunk end: one Ln+Exp on [8,512] -> 1/D, matmul-broadcast to 64
    partitions, DVE mul -> zt bf16 -> output projection -> DMA out.
  - proj/outproj units are interleaved between attention units by a
    build-time greedy scheduler that tracks simulated PE/ACT clocks.
"""
import numpy as np

import concourse.bass as bass
import concourse.tile as tile
import concourse.mybir as mybir
from concourse.vector_clock import ScopedClock
from concourse.bass_utils import run_bass_kernel_spmd

D_MODEL = 1024
D_HEAD = 64
B = 4
T = 2048
H = 8              # heads per core
HG = H * D_HEAD    # 512 head-dim columns per core
TCH = 512          # q/t chunk
NCH = T // TCH     # 4
NDM = D_MODEL // 128  # 8 d_model chunks

F32 = mybir.dt.float32
BF16 = mybir.dt.bfloat16
AF = mybir.ActivationFunctionType


class _TC(tile.TileContext):
    """TileContext whose tail drain carries no sem waits (this walrus build
    rejects >1 sync wait per instruction and any wait on a Drain)."""

    def _drain_and_barrier(self, tick_clock, wait_clock):
        drain_inst = self.nc.sync.drain()
        wait_clock.add_sem_waits(
            drain_inst.ins, ScopedClock({None: tick_clock.global_clock})
        )
        si = drain_inst.ins.sync_info
        waits = list(si.on_wait) if si is not None else []
        if waits:
            drain_inst.ins.sync_info = mybir.SyncInfo(
                on_wait=[], on_update=list(si.on_update)
            )
            for w in waits:
                nop = self.nc.sync.nop(nofuse=True)
                nop.ins.sync_info = mybir.SyncInfo(on_wait=[w], on_update=[])
        self.nc.all_engine_barrier()
        popped = self.nc._tile_sem_poison_stack.pop()
        assert popped is self._sem_poison
        self.nc.clear_and_free_semaphores(list(self.sems.allocated().values()))
        self.nc.all_engine_barrier()


def _split_multi_waits(nc):
    """Move all-but-one sem wait of every instruction onto same-engine NOPs."""
    cnt = 0
    for f in nc.m.functions:
        for b in f.blocks:
            new = []
            for inst in b.instructions:
                si = inst.sync_info
                if si is not None and si.on_wait is not None:
                    waits = list(si.on_wait)
                    max_keep = 0 if inst.opcode == "Drain" else 1
                    if len(waits) > max_keep:
                        keep = waits[len(waits) - max_keep:] if max_keep else []
                        spill = waits[: len(waits) - max_keep]
                        for w in spill:
                            nop = mybir.InstNoOp(
                                name=f"I-wsplit-{cnt}", engine=inst.engine,
                                ins=[], outs=[],
                            )
                            nop.sync_info = mybir.SyncInfo(
                                on_wait=[w], on_update=[]
                            )
                            new.append(nop)
                            cnt += 1
                        inst.sync_info = mybir.SyncInfo(
                            on_wait=keep, on_update=list(si.on_update)
                        )
                new.append(inst)
            b.instructions = new
    return cnt


def _build():
    nc = bass.Bass("TRN2", target_bir_lowering=False)
    xT = nc.dram_tensor("xT", (D_MODEL, T), BF16, kind="ExternalInput")
    wq = nc.dram_tensor("wq", (D_MODEL, HG), BF16, kind="ExternalInput")
    wk = nc.dram_tensor("wk", (D_MODEL, HG), BF16, kind="ExternalInput")
    wv = nc.dram_tensor("wv", (D_MODEL, HG), BF16, kind="ExternalInput")
    wo = nc.dram_tensor("wo", (HG, D_MODEL), BF16, kind="ExternalInput")
    tri = nc.dram_tensor("tri", (128, 128), BF16, kind="ExternalInput")
    sel8 = nc.dram_tensor("sel8", (8, 8 * 64), BF16, kind="ExternalInput")
    sel65 = nc.dram_tensor("sel65", (D_HEAD + 1, 8 * 8), BF16,
                           kind="ExternalInput")
    vones = nc.dram_tensor("vones", (128, T // 128, H, 1), BF16,
                           kind="ExternalInput")
    out = nc.dram_tensor("out", (T, D_MODEL), F32, kind="ExternalOutput")

    from contextlib import ExitStack
    with _TC(nc) as tc, ExitStack() as ctx:
        consts = ctx.enter_context(tc.tile_pool(name="consts", bufs=1))
        xs_pool = ctx.enter_context(tc.tile_pool(name="xs", bufs=3))
        kt_pool = ctx.enter_context(tc.tile_pool(name="kt", bufs=1))
        v_pool = ctx.enter_context(tc.tile_pool(name="v", bufs=1))
        qt_pool = ctx.enter_context(tc.tile_pool(name="qt", bufs=3))
        zt_pool = ctx.enter_context(tc.tile_pool(name="zt", bufs=2))
        et_pool = ctx.enter_context(tc.tile_pool(name="et", bufs=4))
        ue_pool = ctx.enter_context(tc.tile_pool(name="ue", bufs=8))
        sm_pool = ctx.enter_context(tc.tile_pool(name="sm", bufs=6))
        ou_pool = ctx.enter_context(tc.tile_pool(name="ou", bufs=3))
        # PSUM: scores 2x2 banks, U 2x1, proj/outproj/db 1, den8 1 = 8 banks
        ps_s = ctx.enter_context(tc.tile_pool(name="ps_s", bufs=2, space="PSUM"))
        ps_u = ctx.enter_context(tc.tile_pool(name="ps_u", bufs=2, space="PSUM"))
        ps_w = ctx.enter_context(tc.tile_pool(name="ps_w", bufs=1, space="PSUM"))
        ps_d = ctx.enter_context(tc.tile_pool(name="ps_d", bufs=1, space="PSUM"))

        xT_r = xT.ap().rearrange("(c p) t -> p c t", p=128)

        # resident weights / constants (wq/wk + first x chunk lead: they gate
        # the first matmuls)
        wq_sb = consts.tile([128, NDM, HG], BF16)
        xs0 = xs_pool.tile([128, NDM, TCH], BF16, name="xs", tag="xs")
        wk_sb = consts.tile([128, NDM, HG], BF16)
        wv_sb = consts.tile([128, NDM, HG], BF16)
        # spread the head DMAs over the three DMA-capable queues; xs0 on two
        # rings, wq/wk column-major per head-pair so each uq(dqc)/uk(dqc)
        # only waits for its own slice
        wq_r = wq.ap().rearrange("(c p) n -> p c n", p=128)
        wk_r = wk.ap().rearrange("(c p) n -> p c n", p=128)
        wv_r = wv.ap().rearrange("(c p) n -> p c n", p=128)
        for c in range(NDM):
            eng = nc.sync if c % 2 == 0 else nc.scalar
            eng.dma_start(out=xs0[:, c, :], in_=xT_r[:, c, 0:TCH])
        # weight slices balanced over all three rings in the order the
        # upfront proj stream consumes them (q0,k0,q1,k1,q2,k2,q3,k3,v*)
        def wdma(eng, t, r, dqc):
            cs = slice(dqc * 128, (dqc + 1) * 128)
            eng.dma_start(out=t[:, :, cs], in_=r[:, :, cs])
        for dqc in (0, 1):
            wdma(nc.gpsimd, wq_sb, wq_r, dqc)
            wdma(nc.gpsimd, wk_sb, wk_r, dqc)
        wdma(nc.sync, wq_sb, wq_r, 2)
        wdma(nc.sync, wk_sb, wk_r, 2)
        wdma(nc.scalar, wq_sb, wq_r, 3)
        wdma(nc.scalar, wk_sb, wk_r, 3)
        for c in range(NDM):
            eng = (nc.gpsimd if c < 4 else
                   (nc.sync if c < 6 else nc.scalar))
            eng.dma_start(out=wv_sb[:, c, :], in_=wv_r[:, c, :])
        tri_sb = consts.tile([128, 128], BF16)
        nc.sync.dma_start(out=tri_sb, in_=tri.ap())
        sel8_sb = consts.tile([8, 8 * 64], BF16)
        nc.sync.dma_start(out=sel8_sb, in_=sel8.ap())
        sel65_sb = consts.tile([D_HEAD + 1, 8, 8], BF16)
        nc.sync.dma_start(out=sel65_sb,
                          in_=sel65.ap().rearrange("p (g m) -> p g m", g=8))
        wo_sb = consts.tile([128, HG // 128, D_MODEL], BF16)
        nc.gpsimd.dma_start(out=wo_sb,
                            in_=wo.ap().rearrange("(c p) n -> p c n", p=128))
        # per-chunk K^T tiles [pair-packed 128, pair, t-in-chunk] and V tiles
        # (V has a ones column so row 64 of U accumulates the denominator)
        kt_tiles = [kt_pool.tile([128, 4, TCH], BF16, name=f"kt{i}", tag=f"kt{i}")
                    for i in range(NCH)]
        v_tiles = [v_pool.tile([128, 4, H, D_HEAD + 1], BF16, name=f"v{i}",
                               tag=f"v{i}") for i in range(NCH)]
        vo_r = vones.ap().rearrange("p (a b) h o -> p a b h o", b=4)
        for i in range(NCH):
            nc.sync.dma_start(out=v_tiles[i][:, :, :, D_HEAD:], in_=vo_r[:, i])

        # ---------------- unit definitions ----------------
        # Each unit is (cost_pe_ns, cost_act_ns, emit_fn). The scheduler
        # tracks simulated engine clocks and interleaves fill (proj/outproj)
        # between attention units so the PE rides just behind ACT.

        # Fill units are emitted as ~430ns granules (2 matmuls each) so the
        # scheduler can drop them into the short PE holes between attention
        # units.  Granule tag: ('proj', ch, sub, idx) / ('outproj', ch, ...).
        def proj_units(ch, xs, qt_sb):
            units = []
            for dqc in range(4):
                st = {}
                def g0(dqc=dqc, st=st):
                    st['pq'] = ps_w.tile([128, TCH], F32, tag="ps_w", name="pq")
                    for c in range(2):
                        nc.tensor.matmul(
                            st['pq'], lhsT=wq_sb[:, c, dqc * 128:(dqc + 1) * 128],
                            rhs=xs[:, c, :], start=(c == 0), stop=False)
                def gmid(c0, dqc=dqc, st=st):
                    for c in range(c0, c0 + 2):
                        nc.tensor.matmul(
                            st['pq'], lhsT=wq_sb[:, c, dqc * 128:(dqc + 1) * 128],
                            rhs=xs[:, c, :], start=False, stop=False)
                def g3(dqc=dqc, st=st):
                    for c in range(6, NDM):
                        nc.tensor.matmul(
                            st['pq'], lhsT=wq_sb[:, c, dqc * 128:(dqc + 1) * 128],
                            rhs=xs[:, c, :], start=False, stop=(c == NDM - 1))
                    nc.vector.tensor_copy(out=qt_sb[:, dqc, :], in_=st['pq'])
                units += [('proj', ch, 'q', dqc, 440, g0, 'a'),
                          ('proj', ch, 'q', dqc, 440, lambda c0=2, g=gmid: g(c0), 'm'),
                          ('proj', ch, 'q', dqc, 440, lambda c0=4, g=gmid: g(c0), 'm'),
                          ('proj', ch, 'q', dqc, 470, g3, 'e')]
            for dqc in range(4):
                st = {}
                def k0(dqc=dqc, st=st):
                    st['pk'] = ps_w.tile([128, TCH], F32, tag="ps_w", name="pk")
                    for c in range(2):
                        nc.tensor.matmul(
                            st['pk'], lhsT=wk_sb[:, c, dqc * 128:(dqc + 1) * 128],
                            rhs=xs[:, c, :], start=(c == 0), stop=False)
                def kmid(c0, dqc=dqc, st=st):
                    for c in range(c0, c0 + 2):
                        nc.tensor.matmul(
                            st['pk'], lhsT=wk_sb[:, c, dqc * 128:(dqc + 1) * 128],
                            rhs=xs[:, c, :], start=False, stop=False)
                def k3(ch=ch, dqc=dqc, st=st):
                    for c in range(6, NDM):
                        nc.tensor.matmul(
                            st['pk'], lhsT=wk_sb[:, c, dqc * 128:(dqc + 1) * 128],
                            rhs=xs[:, c, :], start=False, stop=(c == NDM - 1))
                    nc.vector.tensor_copy(out=kt_tiles[ch][:, dqc, :],
                                          in_=st['pk'])
                units += [('proj', ch, 'k', dqc, 440, k0, 'a'),
                          ('proj', ch, 'k', dqc, 440, lambda c0=2, g=kmid: g(c0), 'm'),
                          ('proj', ch, 'k', dqc, 440, lambda c0=4, g=kmid: g(c0), 'm'),
                          ('proj', ch, 'k', dqc, 470, k3, 'e')]
            for tt in range(4):
                st = {}
                def v0(tt=tt, st=st):
                    st['pv'] = ps_w.tile([128, HG], F32, tag="ps_w", name="pv")
                    for c in range(2):
                        nc.tensor.matmul(
                            st['pv'], lhsT=xs[:, c, tt * 128:(tt + 1) * 128],
                            rhs=wv_sb[:, c, :], start=(c == 0), stop=False)
                def vmid(c0, tt=tt, st=st):
                    for c in range(c0, c0 + 2):
                        nc.tensor.matmul(
                            st['pv'], lhsT=xs[:, c, tt * 128:(tt + 1) * 128],
                            rhs=wv_sb[:, c, :], start=False, stop=False)
                def v3(ch=ch, tt=tt, st=st):
                    for c in range(6, NDM):
                        nc.tensor.matmul(
                            st['pv'], lhsT=xs[:, c, tt * 128:(tt + 1) * 128],
                            rhs=wv_sb[:, c, :], start=False, stop=(c == NDM - 1))
                    nc.vector.tensor_copy(
                        out=v_tiles[ch][:, tt, :, 0:D_HEAD],
                        in_=st['pv'].rearrange("p (h d) -> p h d", h=H))
                units += [('proj', ch, 'v', tt, 440, v0, 'a'),
                          ('proj', ch, 'v', tt, 440, lambda c0=2, g=vmid: g(c0), 'm'),
                          ('proj', ch, 'v', tt, 440, lambda c0=4, g=vmid: g(c0), 'm'),
                          ('proj', ch, 'v', tt, 470, v3, 'e')]
            return units

        def outproj_units(ch, zt_sb, scalar_evict=False, alt_pool=False):
            units = []
            q0 = ch * TCH
            for tt in range(4):
                st = {}
                def o_alloc(st=st):
                    st['o'] = ou_pool.tile([128, D_MODEL], F32, name="o_sb")
                def mk_half(dc, tt=tt, st=st):
                    def first(dc=dc, tt=tt, st=st):
                        pool, tag = ((ps_d, "den8")
                                     if alt_pool and (tt * 2 + dc) % 2
                                     else (ps_w, "ps_w"))
                        st['po'] = pool.tile([128, 512], F32, tag=tag,
                                             name="po")
                        for kc in range(2):
                            nc.tensor.matmul(
                                st['po'],
                                lhsT=zt_sb[:, kc, tt * 128:(tt + 1) * 128],
                                rhs=wo_sb[:, kc, dc * 512:(dc + 1) * 512],
                                start=(kc == 0), stop=False)
                    def second(dc=dc, tt=tt, st=st):
                        for kc in range(2, 4):
                            nc.tensor.matmul(
                                st['po'],
                                lhsT=zt_sb[:, kc, tt * 128:(tt + 1) * 128],
                                rhs=wo_sb[:, kc, dc * 512:(dc + 1) * 512],
                                start=False, stop=(kc == 3))
                        if scalar_evict:
                            nc.scalar.activation(
                                out=st['o'][:, dc * 512:(dc + 1) * 512],
                                in_=st['po'], func=AF.Copy)
                        else:
                            nc.vector.tensor_copy(
                                out=st['o'][:, dc * 512:(dc + 1) * 512],
                                in_=st['po'])
                        if dc == 1:
                            r0 = q0 + tt * 128
                            eng = nc.sync if tt % 2 == 0 else nc.gpsimd
                            eng.dma_start(out=out.ap()[r0:r0 + 128, :],
                                          in_=st['o'])
                    return first, second
                f0, s0 = mk_half(0)
                f1, s1 = mk_half(1)
                def g00(f=f0, oa=o_alloc):
                    oa(); f()
                units += [('outproj', ch, 'o', tt, 440, g00, 'a'),
                          ('outproj', ch, 'o', tt, 470, s0, 'e'),
                          ('outproj', ch, 'o', tt, 440, f1, 'a'),
                          ('outproj', ch, 'o', tt, 470, s1, 'e')]
            return units

        # ---- attention row (chunk ch, head pair hp) ----
        # units tagged ('S', kb) / ('AV', kb) / ('EV', None) so the emission
        # loop can model the exp(kb) -> AV(kb) gating.
        def att_row_units(ch, hp, qt_sb, uev, den8_ps, row_state):
            nkb = 4 * ch + 4
            ets = [None] * nkb
            u_ps = [None, None]

            def mk_S(kb):
                j = kb - 4 * ch
                ca = 128 * j if j > 0 else 0
                ncols = TCH - ca
                def S(kb=kb, ca=ca, j=j):
                    s2 = ps_s.tile([128, 2, TCH], F32, name="s2", tag="s2")
                    kt_t = kt_tiles[kb // 4]
                    oa = (kb % 4) * 128
                    for par in range(2):
                        p0, p1 = 64 * par, 64 * par + 64
                        nc.tensor.matmul(
                            s2[:, par, ca:],
                            lhsT=kt_t[p0:p1, hp, oa:oa + 128],
                            rhs=qt_sb[p0:p1, hp, ca:],
                            start=True, stop=True,
                            tile_position=(64 * par, 0))
                    et = et_pool.tile([128, 2, TCH], BF16, name="et", tag="et")
                    nc.scalar.activation(out=et[:, :, ca:], in_=s2[:, :, ca:],
                                         func=AF.Exp, scale=0.125)
                    if j >= 0:
                        for par in range(2):
                            nc.vector.tensor_mul(et[:, par, ca:ca + 128],
                                                 et[:, par, ca:ca + 128],
                                                 tri_sb)
                    ets[kb] = et
                # PE: the two score MMs run concurrently (row split)
                return ('S', kb, int(ncols / 2.4) + 15,
                        int((2 * ncols + 352) / 1.2) + 60, S)

            def mk_AV(kb):
                j = kb - 4 * ch
                ca = 128 * j if j > 0 else 0
                ncols = TCH - ca
                def AV(kb=kb, ca=ca):
                    if kb == 0:
                        u_ps[0] = ps_u.tile([D_HEAD + 1, TCH], F32, name="u_ps",
                                            tag="u_ps")
                        u_ps[1] = ps_u.tile([D_HEAD + 1, TCH], F32, name="u_ps",
                                            tag="u_ps")
                    et = ets[kb]
                    for par in range(2):
                        h = 2 * hp + par
                        nc.tensor.matmul(
                            u_ps[par][:, ca:],
                            lhsT=v_tiles[kb // 4][:, kb % 4, h, :],
                            rhs=et[:, par, ca:],
                            start=(kb == 0), stop=(kb == nkb - 1))
                    ets[kb] = None
                return ('AV', kb, 2 * int(ncols / 2.4) + 25, 0, AV)

            def evict():
                # U psum -> SBUF bf16 (frees the 2 psum banks for next row),
                # then matmul-gather this row's denominators (uev row 64).
                # ACT-copy in ch0/1 keeps the DVE queue clear of the next
                # row's gating tri-mask.
                for par in range(2):
                    nc.vector.tensor_copy(out=uev[:, par, :], in_=u_ps[par])
                for par in range(2):
                    g = 2 * hp + par
                    nc.tensor.matmul(den8_ps, lhsT=sel65_sb[:, g, :],
                                     rhs=uev[:, par, :],
                                     start=(g == 0), stop=(g == 7))

            units = []
            units.append(mk_S(0))
            for kb in range(1, nkb):
                units.append(mk_S(kb))
                units.append(mk_AV(kb - 1))
            units.append(mk_AV(nkb - 1))
            units.append(('EV', None, 450, 0, evict))
            return units

        def div_units(ch, uevs, den8_ps, zt_sb):
            """Chunk end: 1/D for all 8 heads, broadcast, zt = U * (1/D)."""
            units = []
            lnd = sm_pool.tile([8, TCH], F32, name="lnd")
            rcp8 = sm_pool.tile([8, TCH], BF16, name="rcp8")
            def u_recip():
                nc.scalar.activation(out=lnd, in_=den8_ps, func=AF.Ln)
                nc.scalar.activation(out=rcp8, in_=lnd, func=AF.Exp,
                                     scale=-1.0)
            units.append((0, 1500, u_recip))
            for hp in range(4):
                def u_div(hp=hp):
                    # separate banks per par (ps_w / ps_d alternate) so the
                    # broadcast of one par overlaps the DVE mul of the other
                    dbs = []
                    for par in range(2):
                        g = 2 * hp + par
                        pool, tag = ((ps_w, "ps_w") if par == 0
                                     else (ps_d, "den8"))
                        db = pool.tile([64, TCH], F32, tag=tag, name="db")
                        nc.tensor.matmul(db,
                                         lhsT=sel8_sb[:, g * 64:(g + 1) * 64],
                                         rhs=rcp8, start=True, stop=True)
                        dbs.append(db)
                    for par in range(2):
                        nc.vector.tensor_mul(
                            zt_sb[64 * par:64 * par + 64, hp, :],
                            uevs[hp][0:D_HEAD, par, :], dbs[par])
                units.append((450, 0, u_div))
            return units

        # ---------------- schedule ----------------
        qt_tiles = [None] * NCH
        xs_tiles = [xs0] + [None] * (NCH - 1)
        zt_tiles = [None] * NCH

        def stage_proj(ch):
            if ch > 0:
                xs_tiles[ch] = xs_pool.tile([128, NDM, TCH], BF16, name="xs",
                                            tag="xs")
                nc.sync.dma_start(
                    out=xs_tiles[ch],
                    in_=xT_r[:, :, ch * TCH:(ch + 1) * TCH])
            qt_tiles[ch] = qt_pool.tile([128, 4, TCH], BF16, name="qt",
                                        tag="qt")
            return proj_units(ch, xs_tiles[ch], qt_tiles[ch])

        # Global fill queue of granules.  'proj' granules required by an
        # attention unit are force-emitted (with their queue prefix) just
        # before it; the rest drop into PE holes (AV exp-waits) or drain
        # when the PE trails ACT.  The scheduler may only pause the queue
        # when no ps_w psum slot is held open by a partially-emitted unit.
        fillq = []
        div_done = [False] * NCH
        pe_t = [0.0]
        act_t = [0.0]
        psw_open = [False]

        def emit_one(ent):
            ent[5]()
            pe_t[0] += ent[4]
            if ent[6] == 'a':
                psw_open[0] = True
            elif ent[6] == 'e':
                psw_open[0] = False

        def emit_fill(gate_t):
            while fillq:
                ent = fillq[0]
                if not psw_open[0] and \
                        pe_t[0] + ent[4] > max(gate_t + 350, act_t[0] + 350):
                    break
                fillq.pop(0)
                emit_one(ent)

        def force_fill(pred):
            """Emit the queue prefix up to the last entry matching pred."""
            last = -1
            for i, ent in enumerate(fillq):
                if pred(ent):
                    last = i
            for ent in fillq[:last + 1]:
                emit_one(ent)
            del fillq[:last + 1]

        def drain_psw():
            while fillq and psw_open[0]:
                emit_one(fillq.pop(0))

        # chunk 0 projections up front, ordered q0,k0 first so the first
        # scores/exp fire as soon as their DMA slices land
        p0 = stage_proj(0)
        def _k0(e):
            if e[2] == 'v':
                return 100 + e[3]          # v last (waits on the wv DMA)
            return e[3] * 2 + (e[2] == 'k')
        for ent in sorted(p0, key=_k0):
            emit_one(ent)

        for ch in range(NCH):
            last = (ch == NCH - 1)
            zt_tiles[ch] = zt_pool.tile([128, 4, TCH], BF16, name="zt",
                                        tag="zt")
            den8_ps = ps_d.tile([8, TCH], F32, name="den8", tag="den8")
            uevs = []
            # queue next chunk's projections as fill
            if ch + 1 < NCH:
                for ent in stage_proj(ch + 1):
                    fillq.append(ent)
            for hp in range(4):
                uev = ue_pool.tile([D_HEAD + 1, 2, TCH], BF16, name="uev",
                                   tag="uev")
                uevs.append(uev)
                exp_done = {}
                row_state = {}
                force_fill(lambda e, hp=hp, ch=ch: e[0] == 'proj'
                           and e[2] == 'q' and e[1] == ch and e[3] == hp)
                for tag, kb, cost_pe, cost_act, fn in att_row_units(
                        ch, hp, qt_tiles[ch], uev, den8_ps, row_state):
                    if tag == 'S':
                        force_fill(lambda e, kb=kb, hp=hp: e[0] == 'proj'
                                   and e[2] == 'k' and e[1] == kb // 4
                                   and e[3] == hp)
                        gate = exp_done.get(kb - 2, 0.0)   # s2 pool depth 2
                    elif tag == 'AV':
                        force_fill(lambda e, kb=kb: e[0] == 'proj'
                                   and e[2] == 'v' and e[1] == kb // 4
                                   and e[3] == kb % 4)
                        gate = exp_done.get(kb, 0.0)
                    else:
                        gate = 0.0
                    emit_fill(gate)
                    fn()
                    if tag == 'S':
                        pe_t[0] = max(pe_t[0], gate) + cost_pe
                        act_t[0] = max(act_t[0], pe_t[0] + 250) + cost_act
                        exp_done[kb] = act_t[0]
                    elif tag == 'AV':
                        pe_t[0] = max(pe_t[0], gate + 250) + cost_pe
                    else:
                        pe_t[0] += cost_pe
            drain_psw()
            for _, cost_act, fn in div_units(ch, uevs, den8_ps,
                                             zt_tiles[ch]):
                emit_fill(pe_t[0])
                drain_psw()
                fn()
                if cost_act:
                    act_t[0] = max(act_t[0], pe_t[0]) + cost_act
                else:
                    pe_t[0] += 450
            div_done[ch] = True
            for ent in outproj_units(ch, zt_tiles[ch], scalar_evict=last,
                                     alt_pool=last):
                fillq.append(ent)
        # drain remaining fill (tail outprojs)
        for ent in list(fillq):
            emit_one(ent)

    _split_multi_waits(nc)
    return nc


_NC_CACHE = None


def _get_nc():
    global _NC_CACHE
    if _NC_CACHE is None:
        _NC_CACHE = _build()
    return _NC_CACHE


def _make_in_maps(x, W_Q, W_K, W_V, W_O):
    x = np.asarray(x, dtype=np.float32)
    W_Q = np.asarray(W_Q, dtype=np.float32)
    W_K = np.asarray(W_K, dtype=np.float32)
    W_V = np.asarray(W_V, dtype=np.float32)
    W_O = np.asarray(W_O, dtype=np.float32)

    import ml_dtypes
    bf = ml_dtypes.bfloat16
    tri = np.triu(np.ones((128, 128), dtype=bf))  # col >= row
    sel8 = np.zeros((8, 8 * 64), dtype=bf)
    for g in range(8):
        sel8[g, g * 64:(g + 1) * 64] = 1.0
    sel65 = np.zeros((D_HEAD + 1, 8 * 8), dtype=bf)
    for g in range(8):
        sel65[D_HEAD, g * 8 + g] = 1.0
    vones = np.ones((128, T // 128, H, 1), dtype=bf)

    in_maps = []
    for core in range(8):
        b, g = core // 2, core % 2
        cs = slice(g * HG, (g + 1) * HG)
        in_maps.append({
            "xT": np.ascontiguousarray(x[b].T).astype(bf),
            "wq": np.ascontiguousarray(W_Q[:, cs]).astype(bf),
            "wk": np.ascontiguousarray(W_K[:, cs]).astype(bf),
            "wv": np.ascontiguousarray(W_V[:, cs]).astype(bf),
            "wo": np.ascontiguousarray(W_O[cs, :]).astype(bf),
            "tri": tri, "sel8": sel8, "sel65": sel65, "vones": vones,
        })
    return in_maps


def kernel(x, W_Q, W_K, W_V, W_O):
    in_maps = _make_in_maps(x, W_Q, W_K, W_V, W_O)
    nc = _get_nc()
    res = run_bass_kernel_spmd(nc, in_maps, core_ids=list(range(8)))
    outs = [res.results[c]["out"] for c in range(8)]
    full = np.stack([outs[2 * b] + outs[2 * b + 1] for b in range(B)], axis=0)
    return full
